# revision 24
# baseline (speedup 1.0000x reference)
"""NF4 dequantization kernel for Trainium2 (8 NeuronCores, tensor-parallel).

Computes: out[g*32+r, n] = nf4_poly(quants[g, r, n]) * scales[g, 0, n]
where nf4_poly is a fixed degree-5 polynomial and quants hold 4-bit codes
(0..15) stored as int32.

Strategy (v2 — "Plan S")
------------------------
The correctness gate is rel_err < 2e-2, so precision is traded for HBM
traffic and engine throughput:

- Host repacks the int32 codes to int8 (lossless, 4x less load traffic)
  and pre-scales the fp32 scales to bf16 `s~ = kappa * s` (bf16 keeps the
  fp32 exponent range, so the tiny kappa costs no precision cliff).
  The output is written fp16 and widened to fp32 on the host.
  Per-core HBM traffic: 8 MiB (q) + 0.5 MiB (s~) + 16 MiB (out) =
  24.5 MiB vs 65 MiB for the all-fp32 variant.

- The quintic p(x) = c5*(x-gamma)*Q1(x)*Q2(x) is split across engines:
    ACT:  v2 = Square(delta*x + delta*A2/2) = delta^2*(x+A2/2)^2
          with delta^2 = 1/K2, so Q2(x)/K2 == v2 + 1 (the +1 rides the
          DVE's free One leaf instead of a scalar slot).
    DVE custom op (one 8-stage pass, the 1x custom-op rate, ~68 us):
          m2 = (x - gamma) * (x^2 + A1*x + B1) * (v2 + One)
    DVE tensor_tensor (fp16 x bf16-broadcast; measured ~4x rate on HW,
    ~0.18 ns/elem):
          out = m2 * s~_broadcast          (kappa = c5*K2 lives in s~)

- Loads go on the SP HWDGE ring, stores on the Pool (gpsimd) ring; the
  ACT engine is kept free for the Square pass (its act table is warmed
  by a 1-element Square before the pipeline starts).

Measured (repeat-slope, 8 cores concurrent): ~95-110 us/core depending
on HBM contention drift (baseline: 212-217 us). Engine budgets: DVE
~85 us busy (custom 68 + TT 17), DMA ~79 us (24.5 MiB at ~310 GB/s),
ACT ~57 us, Pool idle. Relative error vs the fp32 reference: 1.77e-3
(gate: 2e-2; dominated by the bf16 scales stream).
"""

import numpy as np

import concourse.bacc as bacc
import concourse.mybir as mybir
import concourse.tile as tile
import concourse.dve_ops as dve_ops
from concourse.dve_spec import Spec, Src0, Src1, C0, C1, C2, One, sq, lower, _has_src1
from concourse.dve_uop import DveOpSpec

# ---------------------------------------------------------------- constants
# Exact real factorization of the reference quintic (float64 roots):
#   p(x) = C5 (x - GAMMA) (x^2 + A1 x + B1) (x^2 + A2 x + B2)
_GAMMA = 7.08749475940335
_A1, _B1 = -27.553653732708256, 220.05216905674467
_A2, _B2 = -2.8501627403574514, 34.84371770657048
_K2 = _B2 - _A2 * _A2 / 4  # 32.81286079494001
_DELTA = float(1.0 / np.sqrt(_K2))  # 0.1745733524172271
_C5 = 1.82943132356953e-05
_KAPPA = float(_C5 * _K2)  # 6.002887535418984e-4

_NCORES = 8
_G, _GS, _N = 256, 32, 8192          # full input shape
_NS = _N // _NCORES                  # 1024 columns per core
_RS = 8                              # group-rows per tile
_GB = 128                            # groups per partition block
_SPL = 1024                          # TT columns on DVE; rest on Pool
_STORE_ENG = "gpsimd"                # HWDGE ring for output stores
_USE_SREP = False                    # replicate scales to a flat TT operand
_BUFS = (4, 2, 2, 4)                 # q, v, m, o tile-pool depths
_STORE_SPLIT = False                 # alternate stores across two rings
_RC_SIZES = None                     # row-chunk sizes (None -> uniform _RS)
_WARM_ACT = True                     # preload the Square act table early
_OUT_BF16 = True                     # m2/out in bf16 (same-dtype TT is faster)
_OUT_I8 = True                       # store output as int8 fixed-point /127
_I8_SCALE = 127.0


def _register_op(name, spec):
    """Append a custom DVE op to the concourse registry (idempotent)."""
    for op in dve_ops.OPS:
        if op.name == name:
            return op
    row = dve_ops._CUSTOM_DVE_ROW_BASE + len(dve_ops.OPS)
    assert row < 0x20, "custom DVE opcode rows exhausted"
    shas = {
        ver: DveOpSpec(
            name=name, opcode=row, uops=lower(spec, ver=ver), rd1_en=_has_src1(spec)
        ).sha(ver)
        for ver in ("v3", "v4")
    }
    op = dve_ops.DveOp(name, spec, subdim=False, uops_sha=shas)
    dve_ops.OPS.append(op)
    dve_ops.CUSTOM_DVE_SPECS[name] = spec
    dve_ops._SUB_OPCODE_FOR_NAME[name] = row
    return op


def _make_core_op():
    return _register_op(
        "NF4_CORE_ANT",
        Spec(
            body=(Src0 - C0) * (sq(Src0) + Src0 * C1 + C2) * (Src1 + One),
            reference=lambda in0, in1, s0, s1, imm2: (in0 - s0)
            * (in0 * in0 + s1 * in0 + imm2)
            * (in1 + 1.0),
        ),
    )


_NC_CACHE = {}


def _build_module(_repeat=1):
    """Build + compile the per-core Bass module (identical on all cores).

    `_repeat` re-runs the whole loop nest N times over the same data —
    used only by benchmarking to measure marginal per-work time."""
    if _repeat in _NC_CACHE:
        return _NC_CACHE[_repeat]

    core_op = _make_core_op()
    nc = bacc.Bacc(
        "TRN2",
        target_bir_lowering=False,
        debug=False,
        enable_asserts=False,
        num_devices=_NCORES,
    )
    q_d = nc.dram_tensor(
        "quants", [_G, _GS, _NS], mybir.dt.int8, kind="ExternalInput"
    ).ap()
    s_d = nc.dram_tensor(
        "scales", [_G, _NS], mybir.dt.bfloat16, kind="ExternalInput"
    ).ap()
    out_dt = mybir.dt.bfloat16 if _OUT_BF16 else mybir.dt.float16
    dram_dt = mybir.dt.int8 if _OUT_I8 else out_dt
    o_d = nc.dram_tensor(
        "out", [_G, _GS, _NS], dram_dt, kind="ExternalOutput"
    ).ap()

    bias_t = nc.alloc_sbuf_tensor("sq_bias", [128, 1], mybir.dt.float32)
    nc.gpsimd.memset(bias_t.ap(), _DELTA * _A2 / 2)
    if _WARM_ACT:
        # touch Square once so its act-table load overlaps the first DMAs
        warm_t = nc.alloc_sbuf_tensor("act_warm", [128, 1], mybir.dt.float16)
        nc.scalar.activation(
            out=warm_t.ap(), in_=bias_t.ap(),
            func=mybir.ActivationFunctionType.Square,
            bias=bias_t.ap(), scale=_DELTA,
        )

    fd = _RS * _NS
    with tile.TileContext(nc) as tc:
        with (
            tc.tile_pool(name="sc", bufs=2) as sc_pool,
            tc.tile_pool(name="sr", bufs=2) as sr_pool,
            tc.tile_pool(name="q", bufs=_BUFS[0]) as q_pool,
            tc.tile_pool(name="v", bufs=_BUFS[1]) as v_pool,
            tc.tile_pool(name="m", bufs=_BUFS[2]) as m_pool,
            tc.tile_pool(name="o", bufs=_BUFS[3]) as o_pool,
        ):
            for gb in [g for g in range(_G // _GB) for _ in range(_repeat)]:
                gsl = slice(gb * _GB, (gb + 1) * _GB)
                s_t = sc_pool.tile([_GB, _NS], mybir.dt.bfloat16, tag="s")
                nc.sync.dma_start(s_t[:], s_d[gsl, :])
                # replicate s over the RS rows once per block (on the idle
                # Pool engine) so the TT reads a flat stride-1 operand
                if _USE_SREP:
                    s_rep = sr_pool.tile([_GB, fd], mybir.dt.bfloat16, tag="srep")
                    nc.gpsimd.tensor_copy(
                        out=s_rep[:].rearrange("p (r n) -> p r n", r=_RS),
                        in_=s_t[:, None, :].broadcast_to([_GB, _RS, _NS]),
                    )

                rc_sizes = list(_RC_SIZES) if _RC_SIZES else [_RS] * (_GS // _RS)
                assert sum(rc_sizes) == _GS
                r0 = 0
                for rc, rs_i in enumerate(rc_sizes):
                    rsl = slice(r0, r0 + rs_i)
                    r0 += rs_i
                    fd_i = rs_i * _NS
                    qt = q_pool.tile([_GB, fd_i], mybir.dt.int8, tag="q")
                    nc.sync.dma_start(
                        qt[:].rearrange("p (r n) -> p r n", r=rs_i),
                        q_d[gsl, rsl, :],
                    )
                    # ACT: v2 = (delta*x + delta*A2/2)^2  == Q2(x)/K2 - 1
                    vt = v_pool.tile([_GB, fd_i], mybir.dt.float16, tag="v")
                    nc.scalar.activation(
                        out=vt[:],
                        in_=qt[:],
                        func=mybir.ActivationFunctionType.Square,
                        bias=bias_t.ap(),
                        scale=_DELTA,
                    )
                    # DVE custom: m2 = (x-gamma)*Q1(x)*(v2+1)
                    mt = m_pool.tile([_GB, fd_i], out_dt, tag="m")
                    nc.vector._custom_dve(
                        core_op, out=mt[:], in0=qt[:], in1=vt[:],
                        s0=_GAMMA, s1=_A1, imm2=_B1,
                    )
                    # out = m2 * s~ (kappa folded into s~ host-side);
                    # column-split between DVE (2x fp16) and Pool.
                    ot = o_pool.tile([_GB, fd_i], out_dt, tag="o")
                    o3 = ot[:].rearrange("p (r n) -> p r n", r=rs_i)
                    if _USE_SREP:
                        nc.vector.tensor_tensor(
                            out=ot[:], in0=mt[:], in1=s_rep[:],
                            op=mybir.AluOpType.mult,
                        )
                    else:
                        m3 = mt[:].rearrange("p (r n) -> p r n", r=rs_i)
                        s_b = s_t[:, None, :].broadcast_to([_GB, rs_i, _NS])
                        nc.vector.tensor_tensor(
                            out=o3[:, :, :_SPL], in0=m3[:, :, :_SPL],
                            in1=s_b[:, :, :_SPL], op=mybir.AluOpType.mult,
                        )
                        if _SPL < _NS:
                            nc.gpsimd.tensor_tensor(
                                out=o3[:, :, _SPL:], in0=m3[:, :, _SPL:],
                                in1=s_b[:, :, _SPL:], op=mybir.AluOpType.mult,
                            )
                    # store on a ring that doesn't contend with busy engines
                    seng = _STORE_ENG
                    if _STORE_SPLIT and rc % 2 == 1:
                        seng = "scalar"
                    getattr(nc, seng).dma_start(o_d[gsl, rsl, :], o3)

    nc.compile()
    _NC_CACHE[_repeat] = nc
    return nc


def _get_runner():
    """Cached jitted 8-core runner (shard_map over the axon devices)."""
    if "runner" in _NC_CACHE:
        return _NC_CACHE["runner"]

    import jax
    from jax.sharding import Mesh, NamedSharding, PartitionSpec
    from jax.experimental.shard_map import shard_map
    from concourse.bass2jax import _bass_exec_p, install_neuronx_cc_hook

    nc = _build_module()
    install_neuronx_cc_hook()

    in_names, out_names, out_avals, zero_outs = [], [], [], []
    for alloc in nc.m.functions[0].allocations:
        if not isinstance(alloc, mybir.MemoryLocationSet):
            continue
        name = alloc.memorylocations[0].name
        if alloc.kind == "ExternalInput":
            in_names.append(name)
        elif alloc.kind == "ExternalOutput":
            shape = tuple(alloc.tensor_shape)
            dtype = mybir.dt.np(alloc.dtype)
            out_names.append(name)
            out_avals.append(jax.core.ShapedArray(shape, dtype))
            zero_outs.append(np.zeros(shape, dtype))

    def _body(*args):
        return tuple(
            _bass_exec_p.bind(
                *args,
                out_avals=tuple(out_avals),
                in_names=tuple(in_names + out_names),
                out_names=tuple(out_names),
                lowering_input_output_aliases=(),
                sim_require_finite=True,
                sim_require_nnan=True,
                nc=nc,
            )
        )

    devices = jax.devices()[:_NCORES]
    mesh = Mesh(np.asarray(devices), ("core",))
    n_all = len(in_names) + len(out_names)
    sharded = jax.jit(
        shard_map(
            _body,
            mesh=mesh,
            in_specs=(PartitionSpec("core"),) * n_all,
            out_specs=(PartitionSpec("core"),) * len(out_names),
            check_rep=False,
        ),
        keep_unused=True,
    )
    sharding = NamedSharding(mesh, PartitionSpec("core"))
    # output placeholders: written by the NEFF, never read back -> resident
    zeros_dev = [
        jax.device_put(
            np.zeros((_NCORES * z.shape[0], *z.shape[1:]), z.dtype), sharding
        )
        for z in zero_outs
    ]
    runner = (sharded, in_names, out_names, sharding, zeros_dev)
    _NC_CACHE["runner"] = runner
    return runner


def kernel(quants: np.ndarray, scales: np.ndarray, **_) -> np.ndarray:
    quants = np.asarray(quants)
    scales = np.asarray(scales)
    assert quants.shape == (_G, _GS, _N) and scales.shape == (_G, 1, _N)

    import jax

    sharded, in_names, out_names, sharding, zeros_dev = _get_runner()

    q8 = quants.astype(np.int8)                      # codes 0..15, lossless
    s_scale = _KAPPA * _I8_SCALE if _OUT_I8 else _KAPPA
    s_k = (scales[:, 0, :] * np.float32(s_scale)).astype(jax.numpy.bfloat16.dtype)

    per_core = {
        "quants": [
            np.ascontiguousarray(q8[:, :, i * _NS : (i + 1) * _NS])
            for i in range(_NCORES)
        ],
        "scales": [
            np.ascontiguousarray(s_k[:, i * _NS : (i + 1) * _NS])
            for i in range(_NCORES)
        ],
        "partition_id": [
            np.array([[i]], dtype=np.uint32) for i in range(_NCORES)
        ],
    }
    args = [
        jax.device_put(np.concatenate(per_core[name], axis=0), sharding)
        for name in in_names
    ]
    outs = sharded(*args, *zeros_dev)
    out = np.asarray(outs[out_names.index("out")])  # [8*256, 32, 1024]
    # reassemble: core-shards on axis 0 -> columns of the full matrix
    full = (
        out.reshape(_NCORES, _G * _GS, _NS)
        .transpose(1, 0, 2)
        .reshape(_G * _GS, _N)
        .astype(np.float32)
    )
    if _OUT_I8:
        # decode the fixed-point transport format (value = i8 / 127)
        full *= np.float32(1.0 / _I8_SCALE)
    return full


if __name__ == "__main__":
    rng = np.random.default_rng(0)
    q = rng.integers(0, 16, (_G, _GS, _N)).astype(np.int32)
    s = rng.random((_G, 1, _N)).astype(np.float32)
    out = kernel(quants=q, scales=s)
    print("out", out.shape, out.dtype, out[0, :4])


# revision 26
# speedup vs baseline: 1.0535x; 1.0535x over previous
"""NF4 dequantization kernel for Trainium2 (8 NeuronCores, tensor-parallel).

Computes: out[g*32+r, n] = nf4_poly(quants[g, r, n]) * scales[g, 0, n]
where nf4_poly is a fixed degree-5 polynomial and quants hold 4-bit codes
(0..15) stored as int32.

Strategy (v2 — "Plan S")
------------------------
The correctness gate is rel_err < 2e-2, so precision is traded for HBM
traffic and engine throughput:

- Host repacks the int32 codes to int8 (lossless, 4x less load traffic)
  and pre-scales the fp32 scales to bf16 `s~ = kappa * s` (bf16 keeps the
  fp32 exponent range, so the tiny kappa costs no precision cliff).
  The output is written bf16 and widened to fp32 on the host.
  Per-core HBM traffic: 8 MiB (q) + 0.5 MiB (s~) + 16 MiB (out) =
  24.5 MiB vs 65 MiB for the all-fp32 variant.

- The quintic p(x) = c5*(x-gamma)*Q1(x)*Q2(x) is split across engines:
    ACT:  v2 = Square(delta*x + delta*A2/2) = delta^2*(x+A2/2)^2
          with delta^2 = 1/K2, so Q2(x)/K2 == v2 + 1 (the +1 rides the
          DVE's free One leaf instead of a scalar slot).
    DVE custom op (one 8-stage pass, the 1x custom-op rate, ~58-68 us):
          m2 = (x - gamma) * (x^2 + A1*x + B1) * (v2 + One)
    DVE tensor_tensor (bf16 everywhere; HW runs 2-byte TT at ~4x,
    measured 0.07-0.18 ns/elem):
          out = m2 * s~_broadcast          (kappa = c5*K2 lives in s~)

- Loads go on the SP HWDGE ring, stores on the Pool (gpsimd) ring; the
  ACT engine is kept free for the Square pass (its act table is warmed
  by a 1-element Square before the pipeline starts).

Rejected variants (HW-measured): Pool tensor_tensor offload (gpsimd is
~2 ns/elem and gates the stores), a flat replicated scales operand
(cross-engine dep + extra SBUF stream, +47 us), int8 fixed-point output
via the gpsimd casting DMA (the cast path eats the 8 MiB saving),
store-ring splitting (DMA fabric is the cap, ~310 GB/s/core).

Measured (repeat-slope, 8 cores concurrent): ~100-104 us/core median,
best quiet-HBM rounds 61-77 us (baseline: 212-217 us). Engine budgets:
DVE ~63-70 us busy (custom 58-68 + TT ~5-12), DMA ~79 us nominal,
ACT ~57 us, Pool idle. Relative error vs the fp32 reference: 2.67e-3
(gate: 2e-2; dominated by the bf16 scales and output streams).
"""

import numpy as np

import concourse.bacc as bacc
import concourse.mybir as mybir
import concourse.tile as tile
import concourse.dve_ops as dve_ops
from concourse.dve_spec import Spec, Src0, Src1, C0, C1, C2, One, sq, lower, _has_src1
from concourse.dve_uop import DveOpSpec

# ---------------------------------------------------------------- constants
# Exact real factorization of the reference quintic (float64 roots):
#   p(x) = C5 (x - GAMMA) (x^2 + A1 x + B1) (x^2 + A2 x + B2)
_GAMMA = 7.08749475940335
_A1, _B1 = -27.553653732708256, 220.05216905674467
_A2, _B2 = -2.8501627403574514, 34.84371770657048
_K2 = _B2 - _A2 * _A2 / 4  # 32.81286079494001
_DELTA = float(1.0 / np.sqrt(_K2))  # 0.1745733524172271
_C5 = 1.82943132356953e-05
_KAPPA = float(_C5 * _K2)  # 6.002887535418984e-4

_NCORES = 8
_G, _GS, _N = 256, 32, 8192          # full input shape
_NS = _N // _NCORES                  # 1024 columns per core
_RS = 8                              # group-rows per tile
_GB = 128                            # groups per partition block
_SPL = 1024                          # TT columns on DVE; rest on Pool
_STORE_ENG = "gpsimd"                # HWDGE ring for output stores
_USE_SREP = False                    # replicate scales to a flat TT operand
_BUFS = (4, 2, 2, 4)                 # q, v, m, o tile-pool depths
_STORE_SPLIT = False                 # alternate stores across two rings
_RC_SIZES = None                     # row-chunk sizes (None -> uniform _RS)
_WARM_ACT = True                     # preload the Square act table early
_OUT_BF16 = True                     # m2/out in bf16 (same-dtype TT is faster)
_OUT_I8 = False                      # store output as int8 fixed-point /127
                                     # (measured slower: the casting DMA
                                     # path eats the 8 MiB traffic saving)
_I8_SCALE = 127.0


def _register_op(name, spec):
    """Append a custom DVE op to the concourse registry (idempotent)."""
    for op in dve_ops.OPS:
        if op.name == name:
            return op
    row = dve_ops._CUSTOM_DVE_ROW_BASE + len(dve_ops.OPS)
    assert row < 0x20, "custom DVE opcode rows exhausted"
    shas = {
        ver: DveOpSpec(
            name=name, opcode=row, uops=lower(spec, ver=ver), rd1_en=_has_src1(spec)
        ).sha(ver)
        for ver in ("v3", "v4")
    }
    op = dve_ops.DveOp(name, spec, subdim=False, uops_sha=shas)
    dve_ops.OPS.append(op)
    dve_ops.CUSTOM_DVE_SPECS[name] = spec
    dve_ops._SUB_OPCODE_FOR_NAME[name] = row
    return op


def _make_core_op():
    return _register_op(
        "NF4_CORE_ANT",
        Spec(
            body=(Src0 - C0) * (sq(Src0) + Src0 * C1 + C2) * (Src1 + One),
            reference=lambda in0, in1, s0, s1, imm2: (in0 - s0)
            * (in0 * in0 + s1 * in0 + imm2)
            * (in1 + 1.0),
        ),
    )


_NC_CACHE = {}


def _build_module(_repeat=1):
    """Build + compile the per-core Bass module (identical on all cores).

    `_repeat` re-runs the whole loop nest N times over the same data —
    used only by benchmarking to measure marginal per-work time."""
    if _repeat in _NC_CACHE:
        return _NC_CACHE[_repeat]

    core_op = _make_core_op()
    nc = bacc.Bacc(
        "TRN2",
        target_bir_lowering=False,
        debug=False,
        enable_asserts=False,
        num_devices=_NCORES,
    )
    q_d = nc.dram_tensor(
        "quants", [_G, _GS, _NS], mybir.dt.int8, kind="ExternalInput"
    ).ap()
    s_d = nc.dram_tensor(
        "scales", [_G, _NS], mybir.dt.bfloat16, kind="ExternalInput"
    ).ap()
    out_dt = mybir.dt.bfloat16 if _OUT_BF16 else mybir.dt.float16
    dram_dt = mybir.dt.int8 if _OUT_I8 else out_dt
    o_d = nc.dram_tensor(
        "out", [_G, _GS, _NS], dram_dt, kind="ExternalOutput"
    ).ap()

    bias_t = nc.alloc_sbuf_tensor("sq_bias", [128, 1], mybir.dt.float32)
    nc.gpsimd.memset(bias_t.ap(), _DELTA * _A2 / 2)
    if _WARM_ACT:
        # touch Square once so its act-table load overlaps the first DMAs
        warm_t = nc.alloc_sbuf_tensor("act_warm", [128, 1], mybir.dt.float16)
        nc.scalar.activation(
            out=warm_t.ap(), in_=bias_t.ap(),
            func=mybir.ActivationFunctionType.Square,
            bias=bias_t.ap(), scale=_DELTA,
        )

    fd = _RS * _NS
    with tile.TileContext(nc) as tc:
        with (
            tc.tile_pool(name="sc", bufs=2) as sc_pool,
            tc.tile_pool(name="sr", bufs=2) as sr_pool,
            tc.tile_pool(name="q", bufs=_BUFS[0]) as q_pool,
            tc.tile_pool(name="v", bufs=_BUFS[1]) as v_pool,
            tc.tile_pool(name="m", bufs=_BUFS[2]) as m_pool,
            tc.tile_pool(name="o", bufs=_BUFS[3]) as o_pool,
        ):
            for gb in [g for g in range(_G // _GB) for _ in range(_repeat)]:
                gsl = slice(gb * _GB, (gb + 1) * _GB)
                s_t = sc_pool.tile([_GB, _NS], mybir.dt.bfloat16, tag="s")
                nc.sync.dma_start(s_t[:], s_d[gsl, :])
                # replicate s over the RS rows once per block (on the idle
                # Pool engine) so the TT reads a flat stride-1 operand
                if _USE_SREP:
                    s_rep = sr_pool.tile([_GB, fd], mybir.dt.bfloat16, tag="srep")
                    nc.gpsimd.tensor_copy(
                        out=s_rep[:].rearrange("p (r n) -> p r n", r=_RS),
                        in_=s_t[:, None, :].broadcast_to([_GB, _RS, _NS]),
                    )

                rc_sizes = list(_RC_SIZES) if _RC_SIZES else [_RS] * (_GS // _RS)
                assert sum(rc_sizes) == _GS
                r0 = 0
                for rc, rs_i in enumerate(rc_sizes):
                    rsl = slice(r0, r0 + rs_i)
                    r0 += rs_i
                    fd_i = rs_i * _NS
                    qt = q_pool.tile([_GB, fd_i], mybir.dt.int8, tag="q")
                    nc.sync.dma_start(
                        qt[:].rearrange("p (r n) -> p r n", r=rs_i),
                        q_d[gsl, rsl, :],
                    )
                    # ACT: v2 = (delta*x + delta*A2/2)^2  == Q2(x)/K2 - 1
                    vt = v_pool.tile([_GB, fd_i], mybir.dt.float16, tag="v")
                    nc.scalar.activation(
                        out=vt[:],
                        in_=qt[:],
                        func=mybir.ActivationFunctionType.Square,
                        bias=bias_t.ap(),
                        scale=_DELTA,
                    )
                    # DVE custom: m2 = (x-gamma)*Q1(x)*(v2+1)
                    mt = m_pool.tile([_GB, fd_i], out_dt, tag="m")
                    nc.vector._custom_dve(
                        core_op, out=mt[:], in0=qt[:], in1=vt[:],
                        s0=_GAMMA, s1=_A1, imm2=_B1,
                    )
                    # out = m2 * s~ (kappa folded into s~ host-side);
                    # column-split between DVE (2x fp16) and Pool.
                    ot = o_pool.tile([_GB, fd_i], out_dt, tag="o")
                    o3 = ot[:].rearrange("p (r n) -> p r n", r=rs_i)
                    if _USE_SREP:
                        nc.vector.tensor_tensor(
                            out=ot[:], in0=mt[:], in1=s_rep[:],
                            op=mybir.AluOpType.mult,
                        )
                    else:
                        m3 = mt[:].rearrange("p (r n) -> p r n", r=rs_i)
                        s_b = s_t[:, None, :].broadcast_to([_GB, rs_i, _NS])
                        nc.vector.tensor_tensor(
                            out=o3[:, :, :_SPL], in0=m3[:, :, :_SPL],
                            in1=s_b[:, :, :_SPL], op=mybir.AluOpType.mult,
                        )
                        if _SPL < _NS:
                            nc.gpsimd.tensor_tensor(
                                out=o3[:, :, _SPL:], in0=m3[:, :, _SPL:],
                                in1=s_b[:, :, _SPL:], op=mybir.AluOpType.mult,
                            )
                    # store on a ring that doesn't contend with busy engines
                    seng = _STORE_ENG
                    if _STORE_SPLIT and rc % 2 == 1:
                        seng = "scalar"
                    getattr(nc, seng).dma_start(o_d[gsl, rsl, :], o3)

    nc.compile()
    _NC_CACHE[_repeat] = nc
    return nc


def _get_runner():
    """Cached jitted 8-core runner (shard_map over the axon devices)."""
    if "runner" in _NC_CACHE:
        return _NC_CACHE["runner"]

    import jax
    from jax.sharding import Mesh, NamedSharding, PartitionSpec
    from jax.experimental.shard_map import shard_map
    from concourse.bass2jax import _bass_exec_p, install_neuronx_cc_hook

    nc = _build_module()
    install_neuronx_cc_hook()

    in_names, out_names, out_avals, zero_outs = [], [], [], []
    for alloc in nc.m.functions[0].allocations:
        if not isinstance(alloc, mybir.MemoryLocationSet):
            continue
        name = alloc.memorylocations[0].name
        if alloc.kind == "ExternalInput":
            in_names.append(name)
        elif alloc.kind == "ExternalOutput":
            shape = tuple(alloc.tensor_shape)
            dtype = mybir.dt.np(alloc.dtype)
            out_names.append(name)
            out_avals.append(jax.core.ShapedArray(shape, dtype))
            zero_outs.append(np.zeros(shape, dtype))

    def _body(*args):
        return tuple(
            _bass_exec_p.bind(
                *args,
                out_avals=tuple(out_avals),
                in_names=tuple(in_names + out_names),
                out_names=tuple(out_names),
                lowering_input_output_aliases=(),
                sim_require_finite=True,
                sim_require_nnan=True,
                nc=nc,
            )
        )

    devices = jax.devices()[:_NCORES]
    mesh = Mesh(np.asarray(devices), ("core",))
    n_all = len(in_names) + len(out_names)
    sharded = jax.jit(
        shard_map(
            _body,
            mesh=mesh,
            in_specs=(PartitionSpec("core"),) * n_all,
            out_specs=(PartitionSpec("core"),) * len(out_names),
            check_rep=False,
        ),
        keep_unused=True,
    )
    sharding = NamedSharding(mesh, PartitionSpec("core"))
    # output placeholders: written by the NEFF, never read back -> resident
    zeros_dev = [
        jax.device_put(
            np.zeros((_NCORES * z.shape[0], *z.shape[1:]), z.dtype), sharding
        )
        for z in zero_outs
    ]
    runner = (sharded, in_names, out_names, sharding, zeros_dev)
    _NC_CACHE["runner"] = runner
    return runner


def kernel(quants: np.ndarray, scales: np.ndarray, **_) -> np.ndarray:
    quants = np.asarray(quants)
    scales = np.asarray(scales)
    assert quants.shape == (_G, _GS, _N) and scales.shape == (_G, 1, _N)

    import jax

    sharded, in_names, out_names, sharding, zeros_dev = _get_runner()

    q8 = quants.astype(np.int8)                      # codes 0..15, lossless
    s_scale = _KAPPA * _I8_SCALE if _OUT_I8 else _KAPPA
    s_k = (scales[:, 0, :] * np.float32(s_scale)).astype(jax.numpy.bfloat16.dtype)

    per_core = {
        "quants": [
            np.ascontiguousarray(q8[:, :, i * _NS : (i + 1) * _NS])
            for i in range(_NCORES)
        ],
        "scales": [
            np.ascontiguousarray(s_k[:, i * _NS : (i + 1) * _NS])
            for i in range(_NCORES)
        ],
        "partition_id": [
            np.array([[i]], dtype=np.uint32) for i in range(_NCORES)
        ],
    }
    args = [
        jax.device_put(np.concatenate(per_core[name], axis=0), sharding)
        for name in in_names
    ]
    outs = sharded(*args, *zeros_dev)
    out = np.asarray(outs[out_names.index("out")])  # [8*256, 32, 1024]
    # reassemble: core-shards on axis 0 -> columns of the full matrix
    full = (
        out.reshape(_NCORES, _G * _GS, _NS)
        .transpose(1, 0, 2)
        .reshape(_G * _GS, _N)
        .astype(np.float32)
    )
    if _OUT_I8:
        # decode the fixed-point transport format (value = i8 / 127)
        full *= np.float32(1.0 / _I8_SCALE)
    return full


if __name__ == "__main__":
    rng = np.random.default_rng(0)
    q = rng.integers(0, 16, (_G, _GS, _N)).astype(np.int32)
    s = rng.random((_G, 1, _N)).astype(np.float32)
    out = kernel(quants=q, scales=s)
    print("out", out.shape, out.dtype, out[0, :4])
